# revision 8
# baseline (speedup 1.0000x reference)
"""NetVLAD++ (nn_NetVLADpp) Trainium2 Bass kernel — 8-core SPMD.

Math simplification
-------------------
reference computes, per (b, t):
    logits = x @ conv_w.T + conv_b ; assn = softmax(logits)        (B,T,K)
    v[k,:] = (x - c[k]) * assn[k]                                   (K,D)
    v      = l2norm(v, axis=D); v = l2norm(v.reshape(K*D))
Since assn > 0 (softmax) and ||v_k|| = assn_k * ||x - c_k||  with
||x - c_k|| ~ 30 >> eps=1e-12, the first l2norm cancels assn exactly:
    v_k = (x - c_k) / ||x - c_k||            (unit vectors)
and the second l2norm divides by sqrt(K) = 8.  So:
    out[t, k, :] = (x[t] - c[k]) / (8 * ||x[t] - c[k]||)
conv_w / conv_b / softmax drop out of the computation entirely.
(Verified numerically vs the jax reference: rel err ~5e-6.)

Kernel strategy (per core; x sharded 8-way over B*T=4096 rows -> 512 rows)
--------------------------------------------------------------------------
||x-c||^2 = xx - 2*xc + cc.  s[t,k] := 1/sqrt(64*(xx-2xc+cc)) is the full
output scale (the /8 folded in).

 - stats: xx64 via ACT Square(scale=8) with accum_out;  128*xc via PE
   matmul of host-transposed bf16 tiles;  s = reciprocal(sqrt(...)) +
   one Newton-Raphson step (ACT Sqrt LUT has a loose ULP budget).
 - main loop (4 t-tiles x 64 k): PSUM block (128,512) = x_tile - c[k]
   via three PE matmuls with constant stationary weights:
       mm1: lhsT = -ones(2,128),  rhs = [c_hi[k]; c_lo[k]]  (rank-2
            broadcast of -c[k]; same cost as rank-1 — cost ~ N columns)
       mm2: lhsT = eye(128),      rhs = x_hi tile   (accumulate)
       mm3: lhsT = eye(128),      rhs = x_lo tile   (accumulate)
   x and c are split hi+lo bf16 pairs so the residual x-c is fp32-grade
   (bf16 rounding of the inputs was the dominant error at ~3e-3 absmax).
   fp32 PSUM accumulation.  Evacuate PSUM->SBUF with the per-partition
   fp32 scale s[t,k] applied during the copy, alternating ScalarE
   (activation Copy w/ scale) and VectorE (tensor_scalar_mul).
   Stage KB=8 k-blocks contiguously (128 x 4096 f32 = 2 MiB) per DMA.

The kernel is HBM-write bound: 64 MiB of f32 output per core
(~187 us at ~358 GB/s per-core HBM write bandwidth).

`reps`: wraps the main loop in a Tile For_i that repeats it `reps` times
(body ignores the index; outputs are idempotent).  Used by test.py to
measure per-iteration HW time as a wall-clock delta between a small-reps
and large-reps program — the axon-tunnel dispatch and 0.5 GiB output
transfer costs cancel in the subtraction.
"""

import numpy as np
import ml_dtypes

BF16 = ml_dtypes.bfloat16

B, T, D, K = 2, 2048, 512, 64
NCORES = 8
RPC = (B * T) // NCORES  # 512 rows per core
NT = RPC // 128          # 4 t-tiles per core
KB = 8                   # k-blocks staged per output DMA (128 x 4096 f32 = 2 MiB)

_NC_CACHE = {}


def build_nc(reps: int = 1, timing: bool = False):
    """timing=True: output goes to an Internal DRAM buffer (same on-device
    DMA work) and only a 4-byte dummy ExternalOutput is returned — removes
    the 0.5 GiB host<->device transfers so wall-clock deltas are clean."""
    key = (reps, timing)
    if key in _NC_CACHE:
        return _NC_CACHE[key]

    import concourse.bacc as bacc
    import concourse.tile as tile
    from concourse import mybir

    f32 = mybir.dt.float32
    bf16 = mybir.dt.bfloat16
    AF = mybir.ActivationFunctionType
    ET = mybir.EngineType

    nc = bacc.Bacc("TRN2", target_bir_lowering=False)

    # [p, i*D+d] = x[i*128+p, d]  (hi + lo bf16 split of fp32 x)
    xh = nc.dram_tensor("xh", [128, NT * D], bf16, kind="ExternalInput").ap()
    xl = nc.dram_tensor("xl", [128, NT * D], bf16, kind="ExternalInput").ap()
    # [p, (i*NT+j)*128+t] = x_hi[i*128+t, j*128+p]
    xt = nc.dram_tensor("xt", [128, NT * NT * 128], bf16, kind="ExternalInput").ap()
    # [0|1, k*D+d] = c_hi|c_lo[k, d]
    cb = nc.dram_tensor("cb", [2, K * D], bf16, kind="ExternalInput").ap()
    # [p, j*K+k] = 128*c[k, j*128+p]
    ct = nc.dram_tensor("ct", [128, 4 * K], bf16, kind="ExternalInput").ap()
    # [p, k] = 64*||c_k||^2  (replicated over partitions)
    cck = nc.dram_tensor("cck", [128, K], f32, kind="ExternalInput").ap()
    eye = nc.dram_tensor("eye", [128, 128], bf16, kind="ExternalInput").ap()
    if timing:
        out = nc.dram_tensor("outbuf", [RPC, K * D], f32).ap()  # Internal
        dummy = nc.dram_tensor("out", [1, 1], f32, kind="ExternalOutput").ap()
    else:
        out = nc.dram_tensor("out", [RPC, K * D], f32, kind="ExternalOutput").ap()
        dummy = None

    with tile.TileContext(nc) as tc:
        with tc.tile_pool(name="singles", bufs=1) as singles:
            xh_sb = singles.tile([128, NT * D], bf16)
            nc.sync.dma_start(out=xh_sb[:], in_=xh)
            xl_sb = singles.tile([128, NT * D], bf16)
            nc.sync.dma_start(out=xl_sb[:], in_=xl)
            xt_sb = singles.tile([128, NT * NT * 128], bf16)
            nc.sync.dma_start(out=xt_sb[:], in_=xt)
            cb_sb = singles.tile([2, K * D], bf16)
            nc.sync.dma_start(out=cb_sb[:], in_=cb)
            ct_sb = singles.tile([128, 4 * K], bf16)
            nc.sync.dma_start(out=ct_sb[:], in_=ct)
            cck_sb = singles.tile([128, K], f32)
            nc.sync.dma_start(out=cck_sb[:], in_=cck)
            eye_sb = singles.tile([128, 128], bf16)
            nc.sync.dma_start(out=eye_sb[:], in_=eye)
            negones = singles.tile([2, 128], bf16)
            nc.vector.memset(negones[:], -1.0)

            s_sb = singles.tile([128, NT * K], f32)   # scale s[t,k] per tile
            xx_sb = singles.tile([128, NT], f32)      # 64*sum(x^2) per tile

            # ---- stats: s = 1/sqrt(64*xx - 128*xc + 64*cc) ----
            with tc.tile_pool(name="scratch", bufs=2) as scratch, \
                 tc.tile_pool(name="psum_s", bufs=2, space="PSUM") as psum_s:
                for i in range(NT):
                    xsq = scratch.tile([128, D], f32)
                    # Square(8*x) = 64 x^2; accum_out gives the free-dim sum
                    nc.scalar.activation(
                        out=xsq[:],
                        in_=xh_sb[:, i * D:(i + 1) * D],
                        func=AF.Square,
                        bias=0.0,
                        scale=8.0,
                        accum_out=xx_sb[:, i:i + 1],
                    )
                    pxc = psum_s.tile([128, K], f32)
                    for j in range(4):
                        nc.tensor.matmul(
                            pxc[:],
                            lhsT=xt_sb[:, (i * NT + j) * 128:(i * NT + j + 1) * 128],
                            rhs=ct_sb[:, j * K:(j + 1) * K],
                            start=(j == 0),
                            stop=(j == 3),
                        )
                    lin = scratch.tile([128, K], f32)
                    # 64*cc - 128*xc
                    nc.vector.tensor_tensor(
                        lin[:], cck_sb[:], pxc[:], mybir.AluOpType.subtract
                    )
                    # z = 64*||x-c||^2 = lin + 64*xx
                    z = scratch.tile([128, K], f32)
                    nc.vector.tensor_scalar_add(
                        out=z[:], in0=lin[:], scalar1=xx_sb[:, i:i + 1]
                    )
                    q = scratch.tile([128, K], f32)
                    nc.scalar.activation(out=q[:], in_=z[:], func=AF.Sqrt)
                    s0 = s_sb[:, i * K:(i + 1) * K]
                    nc.vector.reciprocal(out=s0, in_=q[:])
                    # Newton-Raphson polish: s = s0*(1.5 - 0.5*z*s0^2)
                    u = scratch.tile([128, K], f32)
                    nc.vector.tensor_tensor(u[:], s0, s0, mybir.AluOpType.mult)
                    nc.vector.tensor_tensor(u[:], u[:], z[:], mybir.AluOpType.mult)
                    nc.vector.tensor_scalar(
                        out=u[:], in0=u[:], scalar1=-0.5, scalar2=1.5,
                        op0=mybir.AluOpType.mult, op1=mybir.AluOpType.add,
                    )
                    nc.vector.tensor_tensor(s0, s0, u[:], mybir.AluOpType.mult)

            # ---- main loop ----
            with tc.tile_pool(name="psum_m", bufs=8, space="PSUM") as psum_m, \
                 tc.tile_pool(name="stage", bufs=3) as stage_p:

                def main_body(_iv=None):
                    for i in range(NT):
                        xh_i = xh_sb[:, i * D:(i + 1) * D]
                        xl_i = xl_sb[:, i * D:(i + 1) * D]
                        for g in range(K // KB):
                            stg = stage_p.tile([128, KB * D], f32)
                            for kk in range(KB):
                                k = g * KB + kk
                                pb = psum_m.tile([128, D], f32)
                                nc.tensor.matmul(
                                    pb[:], lhsT=negones[:],
                                    rhs=cb_sb[0:2, k * D:(k + 1) * D],
                                    start=True, stop=False,
                                )
                                nc.tensor.matmul(
                                    pb[:], lhsT=eye_sb[:], rhs=xh_i,
                                    start=False, stop=False,
                                )
                                nc.tensor.matmul(
                                    pb[:], lhsT=eye_sb[:], rhs=xl_i,
                                    start=False, stop=True,
                                )
                                scol = s_sb[:, i * K + k: i * K + k + 1]
                                dst = stg[:, kk * D:(kk + 1) * D]
                                if kk % 2 == 0:
                                    nc.scalar.mul(out=dst, in_=pb[:], mul=scol)
                                else:
                                    nc.vector.tensor_scalar_mul(
                                        out=dst, in0=pb[:], scalar1=scol
                                    )
                            nc.sync.dma_start(
                                out=out[i * 128:(i + 1) * 128,
                                        g * KB * D:(g + 1) * KB * D],
                                in_=stg[:],
                            )

                if reps == 1:
                    main_body()
                else:
                    with tc.For_i(
                        0, reps, 1,
                        hint_engines=(ET.PE, ET.DVE, ET.Activation, ET.SP),
                    ) as _i:
                        main_body(_i)

                if dummy is not None:
                    dt_ = stage_p.tile([1, 1], f32)
                    nc.vector.memset(dt_[:], 1.0)
                    nc.sync.dma_start(out=dummy, in_=dt_[:])

    nc.finalize()
    _NC_CACHE[key] = nc
    return nc


def prepare_in_maps(x: np.ndarray, centers: np.ndarray):
    """Shard + lay out host-side inputs for the 8 cores."""
    x = np.ascontiguousarray(np.asarray(x, dtype=np.float32)).reshape(B * T, D)
    c = np.asarray(centers, dtype=np.float32)

    c_hi = c.astype(BF16)
    c_lo = (c - c_hi.astype(np.float32)).astype(BF16)
    cb = np.concatenate(
        [c_hi.reshape(1, K * D), c_lo.reshape(1, K * D)], axis=0
    )  # (2, K*D) bf16
    # ct[p, j*K+k] = 128*c[k, j*128+p]
    ct = np.ascontiguousarray(
        (128.0 * c.T).reshape(4, 128, K).transpose(1, 0, 2).reshape(128, 4 * K)
    ).astype(BF16)
    cck = np.ascontiguousarray(
        np.tile((64.0 * (c * c).sum(axis=1))[None, :], (128, 1))
    ).astype(np.float32)
    eye = np.eye(128, dtype=np.float32).astype(BF16)

    in_maps = []
    for core in range(NCORES):
        shard = x[core * RPC:(core + 1) * RPC]  # (512, 512) f32
        x_hi = shard.astype(BF16)
        x_lo = (shard - x_hi.astype(np.float32)).astype(BF16)

        def part_major(a):  # (512, D) -> (128, NT*D), [p, i*D+d] = a[i*128+p, d]
            return np.ascontiguousarray(
                a.reshape(NT, 128, D).transpose(1, 0, 2).reshape(128, NT * D)
            )

        # xt[p, (i*NT+j)*128+t] = x_hi[i*128+t, j*128+p]
        s5 = x_hi.astype(np.float32).reshape(NT, 128, NT, 128)  # [i, t, j, p]
        xth = np.ascontiguousarray(
            s5.transpose(3, 0, 2, 1).reshape(128, NT * NT * 128)
        ).astype(BF16)
        in_maps.append({
            "xh": part_major(x_hi), "xl": part_major(x_lo), "xt": xth,
            "cb": cb, "ct": ct, "cck": cck, "eye": eye,
        })
    return in_maps


def run(in_maps, reps: int = 1, timing: bool = False, **kwargs):
    from concourse.bass_utils import run_bass_kernel_spmd
    nc = build_nc(reps, timing=timing)
    return run_bass_kernel_spmd(nc, in_maps, core_ids=list(range(NCORES)), **kwargs)


def kernel(x, centers, conv_w=None, conv_b=None):
    """Full-input, full-output NetVLAD++ kernel on 8 NeuronCores."""
    in_maps = prepare_in_maps(x, centers)
    res = run(in_maps, reps=1)
    out = np.concatenate(
        [res.results[i]["out"] for i in range(NCORES)], axis=0
    )
    return np.ascontiguousarray(out.reshape(B, T, K * D).astype(np.float32))
